# revision 11
# baseline (speedup 1.0000x reference)
"""Trainium2 Bass kernel for CrossMerge3D.

Input ys: [B=2, S=12, C=96, 32, 32, 32] f32. For each (b, c):
  out = (m0 + perm_j(m1) + perm_k(m2)) / 12
where, with the 12 scans split into 3 groups of 4, each group combines as
  m_g = s0 + s1 + flip(s2 + s3)   (flip over the flattened 32^3 volume)
and group 1's volume is stored as (j,k,i), group 2's as (k,i,j); perm_j /
perm_k bring them back to (i,j,k).

Sharding: 8 cores = batch (2) x channel quarters (4) -> 24 channels/core.
No cross-core communication. Per-core layout: 4 channels x 32
leading-spatial -> 128 SBUF partitions, 1024-wide free dim.

v3 (v0 was DVE-bound; v1/v2 fixed engine balance but lost time to DMA
trigger/semaphore pressure on the two DGE rings and a long serial tail):
- loads are 3 MiB 6-slice DMAs, one per ring per group (12 load
  triggers total instead of 36) -> far fewer semaphore ops on the SP
  and ACT sequencers, smoother 420 GB/s streaming.
- fwd pair-sums (DVE) write straight into PSUM; the flip matmuls
  accumulate on top (start=False), so each fwd slice pair costs one
  DVE op and zero matmuls. rev pair-sums (GpSimd, which cannot touch
  PSUM) go to f32r SBUF tiles feeding 1-cyc/row fp32r matmuls.
- flip's free-reversal and group 2's (i,j)->(j,i) free permute ride the
  matmul moving-operand APs; group 2's merge is fully matmul-built
  (I@sigma(fC) + J@sigma(rev(rC))) with /12 in its stationaries.
- the /12 for the other two groups rides the two final adds as
  scalar_tensor_tensor: a = ps0/12 + T2, o = T1/12 + a. No standalone
  scale op. ST1 writes its result (k,j)->(j,k) pre-permuted so o is a
  fully contiguous op.
- finals run one group late (emitted before the next group's adds) so
  the in-order DVE stream never blocks, while psum recycling stays
  legal (ps1 bufs=2 fills the 8 PSUM banks with ps0/ps2 at bufs=1).
"""

import numpy as np

_B, _S, _C, _D = 2, 12, 96, 32
_NCORE = 8
_CL = _C // 4          # 24 channels per core
_G = _CL // 4          # 6 macro tiles of 4 channels (128 partitions)
_FREE = _D * _D        # 1024

_nc = None


def _build_program():
    from concourse import bacc, tile, mybir

    f32 = mybir.dt.float32
    f32r = mybir.dt.float32r
    add = mybir.AluOpType.add
    mult = mybir.AluOpType.mult
    nc = bacc.Bacc(
        "TRN2", target_bir_lowering=False, debug=False, num_devices=_NCORE
    )
    ys = nc.dram_tensor("ys", [_S, _CL, _D, _D, _D], f32, kind="ExternalInput")
    out = nc.dram_tensor("out", [_CL, _D, _D, _D], f32, kind="ExternalOutput")
    ysa = ys.ap()
    outa = out.ap()

    with tile.TileContext(nc) as tc:
        with (
            tc.tile_pool(name="const", bufs=1) as cst,
            tc.tile_pool(name="io", bufs=2) as iop,
            tc.tile_pool(name="tmp", bufs=2) as tmp,
            tc.tile_pool(name="ps", bufs=1, space="PSUM") as ps,
        ):
            # stationaries built f32, rounded to f32r (fp32r matmul operands
            # must come from f32r-rounding producers):
            #   J1   = 32-block anti-diagonal (unscaled, ps0/ps1 flips)
            #   I12  = I/12, J12 = J1/12 (group 2's fully-matmul merge)
            jf = cst.tile([128, 128], f32, tag="jf", name="jf")
            nc.gpsimd.memset(jf[:], 1.0)
            for b in range(4):
                nc.gpsimd.affine_select(
                    out=jf[32 * b:32 * b + 32, :],
                    in_=jf[32 * b:32 * b + 32, :],
                    compare_op=mybir.AluOpType.is_equal, fill=0.0,
                    base=-(32 * b + 31), pattern=[[1, 128]],
                    channel_multiplier=1,
                )
            J1 = cst.tile([128, 128], f32r, tag="J1", name="J1")
            nc.vector.tensor_copy(J1[:], jf[:])
            J12 = cst.tile([128, 128], f32r, tag="J12", name="J12")
            nc.vector.tensor_scalar(J12[:], jf[:], 1.0 / 12.0, None, mult)
            if_ = cst.tile([128, 128], f32, tag="if", name="if")
            nc.gpsimd.memset(if_[:], 1.0 / 12.0)
            nc.gpsimd.affine_select(
                out=if_[:], in_=if_[:],
                compare_op=mybir.AluOpType.is_equal, fill=0.0,
                base=0, pattern=[[1, 128]], channel_multiplier=-1,
            )
            I12 = cst.tile([128, 128], f32r, tag="I12", name="I12")
            nc.vector.tensor_copy(I12[:], if_[:])

            finish_prev = None

            for g in range(_G):
                cs = slice(4 * g, 4 * (g + 1))

                def load6(s, tag, eng):
                    # one 3 MiB DMA: 6 scan slices for these 4 channels
                    t = iop.tile([128, 6 * _FREE], f32, tag=tag, name=tag)
                    src = ysa[s:s + 6, cs].rearrange(
                        "s c i j k -> (c i) s (j k)"
                    )
                    dst = t[:].rearrange("p (s f) -> p s f", s=6)
                    eng.dma_start(out=dst, in_=src)
                    return t

                La = load6(0, "La", nc.sync)    # s0,s1 | s2,s3 | s4,s5
                Lb = load6(6, "Lb", nc.scalar)  # s6,s7 | s8,s9 | s10,s11

                def sl(t, n):
                    return t[:, n * _FREE:(n + 1) * _FREE]

                # finals of the previous group: before this group's adds so
                # the psum recycle (ps0/ps2 bufs=1) stays ahead of them in
                # the DVE stream.
                if finish_prev is not None:
                    finish_prev()

                ps0 = ps.tile([128, _FREE], f32, tag="ps0", name="ps0", bufs=1)
                ps1 = ps.tile([128, _FREE], f32, tag="ps1", name="ps1", bufs=2)
                ps2 = ps.tile([128, _FREE], f32, tag="ps2", name="ps2", bufs=1)

                # fwd sums straight into PSUM (DVE); rev sums + group 2's fwd
                # into f32r SBUF tiles for the matmuls
                nc.vector.tensor_tensor(ps0[:], sl(La, 0), sl(La, 1), add)
                nc.vector.tensor_tensor(ps1[:], sl(La, 4), sl(La, 5), add)
                fC = iop.tile([128, _FREE], f32r, tag="fC", name="fC")
                nc.vector.tensor_tensor(fC[:], sl(Lb, 2), sl(Lb, 3), add)
                rA = iop.tile([128, _FREE], f32r, tag="rA", name="rA")
                nc.gpsimd.tensor_tensor(rA[:], sl(La, 2), sl(La, 3), add)
                rB = iop.tile([128, _FREE], f32r, tag="rB", name="rB")
                nc.gpsimd.tensor_tensor(rB[:], sl(Lb, 0), sl(Lb, 1), add)
                rC = iop.tile([128, _FREE], f32r, tag="rC", name="rC")
                nc.gpsimd.tensor_tensor(rC[:], sl(Lb, 4), sl(Lb, 5), add)

                # group 2's free permute (i,j)->(j,i) rides the moving APs
                fC_s = fC[:].rearrange("p (a b) -> p a b", a=_D).transpose(
                    [0, 2, 1]
                )
                rC_s = rC[:].rearrange("p (a b) -> p a b", a=_D)[
                    :, ::-1, ::-1
                ].transpose([0, 2, 1])
                rA_r = rA[:][:, ::-1]
                rB_r = rB[:][:, ::-1]

                for h in (0, 1):
                    hs = slice(512 * h, 512 * h + 512)
                    hb = slice(16 * h, 16 * h + 16)
                    # flips accumulate onto the DVE-written fwd sums
                    nc.tensor.matmul(ps1[:, hs], J1[:], rB_r[:, hs],
                                     start=False, stop=True,
                                     skip_group_check=True)
                    nc.tensor.matmul(ps2[:, hs], I12[:], fC_s[:, hb],
                                     start=True, stop=False)
                    nc.tensor.matmul(ps2[:, hs], J12[:], rC_s[:, hb],
                                     start=False, stop=True)
                    nc.tensor.matmul(ps0[:, hs], J1[:], rA_r[:, hs],
                                     start=False, stop=True,
                                     skip_group_check=True)

                def make_finals(g, ps0, ps1, ps2, cs):
                    def finals():
                        # ps2 holds sigma(m2)/12 at (c,k),(j,i); ST -> (c,i),(j,k)
                        T2 = tmp.tile([128, _FREE], f32, tag="T2", name="T2")
                        nc.vector.transpose(T2[:], ps2[:])
                        # a = ps0/12 + T2 (one stt op; frees ps0)
                        a = tmp.tile([128, _FREE], f32, tag="a", name="a")
                        nc.vector.scalar_tensor_tensor(
                            a[:], ps0[:], 1.0 / 12.0, T2[:], mult, add)
                        # ps1 holds m1 at (c,j),(k,i); ST with (j,k)-permuted
                        # out AP -> T1 already in (c,i),(j,k)
                        T1 = tmp.tile([128, _FREE], f32, tag="T1", name="T1")
                        T1o = T1[:].rearrange(
                            "p (a b) -> p a b", a=_D
                        ).transpose([0, 2, 1])
                        nc.vector.transpose(T1o, ps1[:])
                        # o = T1/12 + a (contiguous stt)
                        o = tmp.tile([128, _FREE], f32, tag="o", name="o")
                        nc.vector.scalar_tensor_tensor(
                            o[:], T1[:], 1.0 / 12.0, a[:], mult, add)
                        nc.sync.dma_start(
                            out=outa[cs].rearrange("c i j k -> (c i) (j k)"),
                            in_=o[:],
                        )
                    return finals

                finish_prev = make_finals(g, ps0, ps1, ps2, cs)

            finish_prev()

    nc.compile()
    return nc


def kernel(ys):
    global _nc
    ys = np.ascontiguousarray(ys, dtype=np.float32)
    assert ys.shape == (_B, _S, _C, _D, _D, _D), ys.shape

    if _nc is None:
        _nc = _build_program()

    from concourse.bass_utils import run_bass_kernel_spmd

    in_maps = []
    for r in range(_NCORE):
        b, q = divmod(r, 4)
        shard = np.ascontiguousarray(ys[b, :, q * _CL:(q + 1) * _CL])
        in_maps.append({"ys": shard})

    res = run_bass_kernel_spmd(_nc, in_maps, list(range(_NCORE)))

    out = np.empty((_B, _C, _D, _D, _D), np.float32)
    for r in range(_NCORE):
        b, q = divmod(r, 4)
        out[b, q * _CL:(q + 1) * _CL] = res.results[r]["out"]

    if res.exec_time_ns is not None:
        print(f"HW exec time: {res.exec_time_ns} ns")
    return out
